# revision 16
# baseline (speedup 1.0000x reference)
"""Block-sparse (block-diagonal, BLOCK=64) multi-head attention for 8 Trainium2 cores.

Sharding: the B*S = 4096 token rows are split into 8 contiguous slices of 512
tokens (attention is block-diagonal with 64-token blocks, so slices at
512-token boundaries are fully independent). Each core runs the whole
projections + attention + output projection for its 512 tokens; weights are
replicated. No collectives; host concatenates the per-core outputs.

Layout strategy (per core):
  - host passes X slices TRANSPOSED (feature-major [1024, 512]) so the kernel
    never has to transpose on-chip; all four weight matrices are DMA'd once
    and stay resident in SBUF (fp16 halves their footprint).
  - Q^T, K^T are produced feature-major [dout, t] (lhsT = W tile, rhs = X^T).
  - V is produced token-major [t, dout]  (lhsT = X^T tile, rhs = W tile).
  - scores for a 128-token chunk: S^T[j, i] = sum_dk K^T[dk, j] Q^T[dk, i]
    (both operands feature-major). Only the two diagonal 64x64 quadrants of
    the [128, 128] psum tile are real in-block scores; exp() is applied to
    just those, and the off-diagonal quadrants of the P tile are zeroed with
    a GpSimd memset (the engine is otherwise idle).
  - row sums r[i]: two ones-vector matmuls replicate the per-head in-block
    column sums into the matching 64-partition strips of a [128, 128] psum
    tile; reciprocal_approx_fast gives 1/r, and the normalization is folded
    into the PSUM->SBUF copy of the attention output (tensor_mul).
  - O^T[dv, i] = V.T @ P with lhsT = V (token-major) -- output is
    feature-major, directly the lhsT of the output projection. The output
    projection for token chunk c is emitted right after attention chunk c, so
    the tensor engine gets dense N=512 matmul work interleaved with the small
    attention matmuls (keeps the HAM clock gate at full rate).

Compute dtype: matmul operands (X, W, Q/K/V, P, O) are stored in fp16 by
default -- 4x matmul throughput vs fp32 and half the DMA bytes, with fp32
PSUM accumulation everywhere. The all-f32 variant is available via
_compute="f32".
"""

import sys

sys.path.insert(0, "/opt/trn_rl_repo")

import numpy as np

N_CORES = 8
B, S, D = 2, 2048, 1024
H, DK = 16, 64
T = (B * S) // N_CORES      # 512 tokens per core
P = 128
KO = D // P                 # 8 contraction tiles
MO = D // P                 # 8 d_out tiles
NC_CHUNKS = T // P          # 4 token chunks per core
HP = H // 2                 # 8 head pairs
NV = D // T                 # 2 output column halves of 512

_cache = {}


def _build_program(compute):
    import concourse.tile as tile
    from concourse import bacc, mybir

    f32 = mybir.dt.float32
    dtc = {"f32": f32, "f16": mybir.dt.float16, "bf16": mybir.dt.bfloat16}[compute]

    nc = bacc.Bacc("TRN2", target_bir_lowering=False, debug=False)

    xq_d = nc.dram_tensor("xq", [D, T], dtc, kind="ExternalInput").ap()
    xk_d = nc.dram_tensor("xk", [D, T], dtc, kind="ExternalInput").ap()
    xv_d = nc.dram_tensor("xv", [D, T], dtc, kind="ExternalInput").ap()
    wq_d = nc.dram_tensor("wq", [MO, P, KO, P], dtc, kind="ExternalInput").ap()
    wk_d = nc.dram_tensor("wk", [MO, P, KO, P], dtc, kind="ExternalInput").ap()
    wv_d = nc.dram_tensor("wv", [D, D], dtc, kind="ExternalInput").ap()
    wo_d = nc.dram_tensor("wo", [D, D], dtc, kind="ExternalInput").ap()
    bq_d = nc.dram_tensor("bq", [P, MO], f32, kind="ExternalInput").ap()
    bk_d = nc.dram_tensor("bk", [P, MO], f32, kind="ExternalInput").ap()
    bv_d = nc.dram_tensor("bv", [D], f32, kind="ExternalInput").ap()
    bo_d = nc.dram_tensor("bo", [D], f32, kind="ExternalInput").ap()
    y_d = nc.dram_tensor("y", [T, D], f32, kind="ExternalOutput").ap()

    with tile.TileContext(nc) as tc:
        with (
            tc.tile_pool(name="singles", bufs=1) as singles,
            tc.tile_pool(name="p2", bufs=4) as p2_pool,
            tc.tile_pool(name="rec", bufs=3) as rec_pool,
            tc.tile_pool(name="ystage", bufs=3) as y_pool,
            tc.tile_pool(name="psproj", bufs=2, space="PSUM") as psproj,
            tc.tile_pool(name="pss", bufs=3, space="PSUM") as pss_pool,
            tc.tile_pool(name="psro", bufs=3, space="PSUM") as psro_pool,
        ):
            # ---- persistent SBUF tensors (inputs as separate per-chunk
            # tiles so each matmul depends only on its own DMA) ----
            xq_t = [singles.tile([P, T], dtc, tag=f"xq{i}", name=f"xq{i}") for i in range(KO)]
            xk_t = [singles.tile([P, T], dtc, tag=f"xk{i}", name=f"xk{i}") for i in range(KO)]
            xv_t = [singles.tile([P, T], dtc, tag=f"xv{i}", name=f"xv{i}") for i in range(KO)]
            wq_t = [singles.tile([P, KO, P], dtc, tag=f"wq{i}", name=f"wq{i}") for i in range(MO)]
            wk_t = [singles.tile([P, KO, P], dtc, tag=f"wk{i}", name=f"wk{i}") for i in range(MO)]
            wv_t = [singles.tile([P, D], dtc, tag=f"wv{i}", name=f"wv{i}") for i in range(KO)]
            wo_t = [singles.tile([P, D], dtc, tag=f"wo{i}", name=f"wo{i}") for i in range(KO)]
            qT_sb = singles.tile([P, MO, T], dtc, tag="qT")
            kT_sb = singles.tile([P, MO, T], dtc, tag="kT")
            v_sb = singles.tile([P, NC_CHUNKS, D], dtc, tag="v")
            oT_sb = singles.tile([P, MO, T], dtc, tag="oT")
            bq_sb = singles.tile([P, MO], f32, tag="bq")
            bk_sb = singles.tile([P, MO], f32, tag="bk")
            bv_sb = singles.tile([P, D], f32, tag="bv")
            bo_sb = singles.tile([P, D], f32, tag="bo")
            ones_sb = singles.tile([P, 64], dtc, tag="ones")

            # inputs arrive as flat [128, *] chunks, alternating issue
            # queues, ordered by first use so the projections start early
            nc.sync.dma_start(bq_sb[:], bq_d[:])
            nc.gpsimd.dma_start(bk_sb[:], bk_d[:])
            for ko in range(KO):
                eng = nc.sync if ko % 2 == 0 else nc.gpsimd
                eng.dma_start(xq_t[ko][:], xq_d[ko * P : (ko + 1) * P, :])
                eng.dma_start(
                    wq_t[ko][:].rearrange("p ko c -> p (ko c)"),
                    wq_d[ko].rearrange("p ko c -> p (ko c)"),
                )
            for ko in range(KO):
                eng = nc.sync if ko % 2 == 0 else nc.gpsimd
                eng.dma_start(xk_t[ko][:], xk_d[ko * P : (ko + 1) * P, :])
                eng.dma_start(
                    wk_t[ko][:].rearrange("p ko c -> p (ko c)"),
                    wk_d[ko].rearrange("p ko c -> p (ko c)"),
                )
            for ko in range(KO):
                eng = nc.sync if ko % 2 == 0 else nc.gpsimd
                eng.dma_start(xv_t[ko][:], xv_d[ko * P : (ko + 1) * P, :])
                eng.dma_start(wv_t[ko][:], wv_d[ko * P : (ko + 1) * P, :])
            nc.sync.dma_start(bv_sb[:], bv_d[None, :].to_broadcast([P, D]))
            for ko in range(KO):
                eng = nc.sync if ko % 2 == 0 else nc.gpsimd
                eng.dma_start(wo_t[ko][:], wo_d[ko * P : (ko + 1) * P, :])
            nc.gpsimd.dma_start(bo_sb[:], bo_d[None, :].to_broadcast([P, D]))

            nc.vector.memset(ones_sb[:], 1.0)

            # PE warm-up: junk matmuls during the DMA lead-in so the HAM
            # clock gate is already at full rate when projections start
            ps_w = psro_pool.tile([64, 64], f32, tag="psro", name="warmup")
            for _ in range(140):
                nc.tensor.matmul(ps_w[:], ones_sb[:, 0:64], ones_sb[:, 0:64],
                                 start=True, stop=True)

            # ---- Q^T / K^T projections (feature-major out) ----
            for w_t, x_t, b_sb, dst in (
                (wq_t, xq_t, bq_sb, qT_sb),
                (wk_t, xk_t, bk_sb, kT_sb),
            ):
                for m in range(MO):
                    ps = psproj.tile([P, T], f32, tag="psproj")
                    for ko in range(KO):
                        nc.tensor.matmul(
                            ps[:],
                            w_t[m][:, ko, :],
                            x_t[ko][:],
                            start=(ko == 0),
                            stop=(ko == KO - 1),
                        )
                    nc.scalar.activation(
                        dst[:, m, :],
                        ps[:],
                        mybir.ActivationFunctionType.Identity,
                        bias=b_sb[:, m : m + 1],
                    )

            # ---- V projection for chunk 0 (later chunks pipeline into the
            # attention loop below to keep the tensor engine dense) ----
            def v_proj_part(mt, part):
                # part 0..7: half n = part//4, contraction pair (2k, 2k+1);
                # one psum tile alive at a time
                n, k2 = part // 4, 2 * (part % 4)
                if part % 4 == 0:
                    v_ps[mt] = psproj.tile(
                        [P, T], f32, tag="psproj", name=f"psv_{mt}_{n}"
                    )
                for ko in (k2, k2 + 1):
                    nc.tensor.matmul(
                        v_ps[mt][:],
                        xv_t[ko][:, mt * P : (mt + 1) * P],
                        wv_t[ko][:, n * T : (n + 1) * T],
                        start=(ko == 0 or ko == k2 == 0 or part % 4 == 0 and ko == k2),
                        stop=(part % 4 == 3 and ko == k2 + 1),
                    )
                if part % 4 == 3:
                    nc.vector.tensor_add(
                        v_sb[:, mt, n * T : (n + 1) * T],
                        v_ps[mt][:],
                        bv_sb[:, n * T : (n + 1) * T],
                    )
                    v_ps[mt] = None

            def y_proj_part(c, part):
                n, k2 = part // 4, 2 * (part % 4)
                if part % 4 == 0:
                    y_ps[c] = psproj.tile(
                        [P, T], f32, tag="psproj", name=f"psy_{c}_{n}"
                    )
                for m in (k2, k2 + 1):
                    nc.tensor.matmul(
                        y_ps[c][:],
                        oT_sb[:, m, c * P : (c + 1) * P],
                        wo_t[m][:, n * T : (n + 1) * T],
                        start=(part % 4 == 0 and m == k2),
                        stop=(part % 4 == 3 and m == k2 + 1),
                    )
                if part % 4 == 3:
                    y_sb = y_pool.tile([P, T], f32, tag="ystage")
                    nc.vector.tensor_add(
                        y_sb[:], y_ps[c][:], bo_sb[:, n * T : (n + 1) * T]
                    )
                    nc.sync.dma_start(
                        y_d[c * P : (c + 1) * P, n * T : (n + 1) * T], y_sb[:]
                    )
                    y_ps[c] = None

            v_ps = [None] * NC_CHUNKS
            y_ps = [None] * NC_CHUNKS
            for part in range(KO):
                v_proj_part(0, part)

            # ---- attention + interleaved output projection, per token chunk ----
            LAG = 5  # iterations between producing an oT tile and consuming it
            iters = [(c, hp) for c in range(NC_CHUNKS) for hp in range(HP)]
            for c in range(NC_CHUNKS):
                tsl = slice(c * P, (c + 1) * P)
                for hp in range(HP):
                    h0, h1 = 2 * hp, 2 * hp + 1
                    p2 = p2_pool.tile([P, 2 * P], dtc, tag="p2")
                    # zero off-diagonal quadrants (exp only writes diagonals)
                    nc.gpsimd.memset(p2[:], 0.0)
                    for idx, h in ((0, h0), (1, h1)):
                        s = slice((h % 2) * 64, (h % 2) * 64 + 64)
                        ps_s = pss_pool.tile([P, P], f32, tag="pss")
                        nc.tensor.matmul(
                            ps_s[:],
                            kT_sb[s, hp, tsl],
                            qT_sb[s, hp, tsl],
                            start=True,
                            stop=True,
                        )
                        for q in (0, 1):
                            qs = slice(q * 64, (q + 1) * 64)
                            nc.scalar.activation(
                                p2[qs, idx * P + q * 64 : idx * P + (q + 1) * 64],
                                ps_s[qs, qs],
                                mybir.ActivationFunctionType.Exp,
                                scale=0.125,
                            )
                    # replicated in-block column sums: head h0 -> partitions
                    # 0:64, head h1 -> partitions 64:128
                    ps_r = psro_pool.tile([P, P], f32, tag="psro")
                    nc.tensor.matmul(
                        ps_r[0:64, :], ones_sb[:], p2[:, 0:P], start=True, stop=True
                    )
                    nc.tensor.matmul(
                        ps_r[64:128, :],
                        ones_sb[:],
                        p2[:, P : 2 * P],
                        start=True,
                        stop=True,
                    )
                    rec = rec_pool.tile([P, P], f32, tag="rec")
                    nc.vector.reciprocal_approx_fast(out=rec[:], in_=ps_r[:])
                    ps_o = psro_pool.tile([P, P], f32, tag="psro")
                    for idx, h in ((0, h0), (1, h1)):
                        nc.tensor.matmul(
                            ps_o[idx * 64 : (idx + 1) * 64, :],
                            v_sb[:, c, h * DK : (h + 1) * DK],
                            p2[:, idx * P : (idx + 1) * P],
                            start=True,
                            stop=True,
                        )
                    nc.vector.tensor_mul(oT_sb[:, hp, tsl], ps_o[:], rec[:])

                    # pipelined dense matmul work: the output-projection
                    # accumulation step for the oT tile produced LAG
                    # iterations ago, and V for chunk c+1
                    k = c * HP + hp
                    if k >= LAG:
                        y_proj_part(*iters[k - LAG])
                    if c + 1 < NC_CHUNKS:
                        v_proj_part(c + 1, hp)

            for k in range(len(iters) - LAG, len(iters)):
                y_proj_part(*iters[k])

    nc.compile()
    return nc


def _get_program(compute):
    if compute not in _cache:
        _cache[compute] = _build_program(compute)
    return _cache[compute]


DEFAULT_COMPUTE = "f16"


def kernel(
    query,
    key,
    value,
    Wq,
    bq,
    Wk,
    bk,
    Wv,
    bv,
    Wo,
    bo,
    _compute=DEFAULT_COMPUTE,
    _trace=False,
):
    from concourse.bass_utils import run_bass_kernel_spmd

    nc = _get_program(_compute)
    if _compute == "bf16":
        import ml_dtypes

        npdt = ml_dtypes.bfloat16
    else:
        npdt = {"f32": np.float32, "f16": np.float16}[_compute]

    def pre_w(w):
        # [din, dout] -> [m, p, ko, c] tiles so each m-tile DMAs contiguously
        return np.ascontiguousarray(
            np.asarray(w, np.float32)
            .reshape(KO, P, MO, P)
            .transpose(2, 1, 0, 3)
            .astype(npdt)
        )

    def pre_b(b):
        return np.ascontiguousarray(np.asarray(b, np.float32).reshape(MO, P).T)

    q2 = np.asarray(query, np.float32).reshape(B * S, D)
    k2 = np.asarray(key, np.float32).reshape(B * S, D)
    v2 = np.asarray(value, np.float32).reshape(B * S, D)
    shared = {
        "wq": pre_w(Wq),
        "wk": pre_w(Wk),
        "wv": np.ascontiguousarray(np.asarray(Wv, np.float32).astype(npdt)),
        "wo": np.ascontiguousarray(np.asarray(Wo, np.float32).astype(npdt)),
        "bq": pre_b(bq),
        "bk": pre_b(bk),
        "bv": np.ascontiguousarray(np.asarray(bv, np.float32)),
        "bo": np.ascontiguousarray(np.asarray(bo, np.float32)),
    }
    in_maps = []
    for c in range(N_CORES):
        rows = slice(c * T, (c + 1) * T)
        in_maps.append(
            {
                "xq": np.ascontiguousarray(q2[rows].T.astype(npdt)),
                "xk": np.ascontiguousarray(k2[rows].T.astype(npdt)),
                "xv": np.ascontiguousarray(v2[rows].T.astype(npdt)),
                **shared,
            }
        )

    kwargs = {}
    if _trace:
        kwargs = {"trace": True}
    res = run_bass_kernel_spmd(nc, in_maps, core_ids=list(range(N_CORES)), **kwargs)
    y = np.concatenate([res.results[c]["y"] for c in range(N_CORES)], axis=0)
    out = y.reshape(B, S, D)
    if _trace:
        return out, res
    return out


# revision 17
# speedup vs baseline: 1.0076x; 1.0076x over previous
"""Block-sparse (block-diagonal, BLOCK=64) multi-head attention for 8 Trainium2 cores.

Sharding: the B*S = 4096 token rows are split into 8 contiguous slices of 512
tokens (attention is block-diagonal with 64-token blocks, so slices at
512-token boundaries are fully independent). Each core runs the whole
projections + attention + output projection for its 512 tokens; weights are
replicated. No collectives; host concatenates the per-core outputs.

Layout strategy (per core):
  - host passes X slices TRANSPOSED (feature-major [1024, 512]) so the kernel
    never has to transpose on-chip; all four weight matrices are DMA'd once
    and stay resident in SBUF (fp16 halves their footprint).
  - Q^T, K^T are produced feature-major [dout, t] (lhsT = W tile, rhs = X^T).
  - V is produced token-major [t, dout]  (lhsT = X^T tile, rhs = W tile).
  - scores for a 128-token chunk: S^T[j, i] = sum_dk K^T[dk, j] Q^T[dk, i]
    (both operands feature-major). Only the two diagonal 64x64 quadrants of
    the [128, 128] psum tile are real in-block scores; exp() is applied to
    just those, and the off-diagonal quadrants of the P tile are zeroed with
    a GpSimd memset (the engine is otherwise idle).
  - row sums r[i]: two ones-vector matmuls replicate the per-head in-block
    column sums into the matching 64-partition strips of a [128, 128] psum
    tile; reciprocal_approx_fast gives 1/r, and the normalization is folded
    into the PSUM->SBUF copy of the attention output (tensor_mul).
  - O^T[dv, i] = V.T @ P with lhsT = V (token-major) -- output is
    feature-major, directly the lhsT of the output projection. The output
    projection for token chunk c is emitted right after attention chunk c, so
    the tensor engine gets dense N=512 matmul work interleaved with the small
    attention matmuls (keeps the HAM clock gate at full rate).

Compute dtype: matmul operands (X, W, Q/K/V, P, O) are stored in fp16 by
default -- 4x matmul throughput vs fp32 and half the DMA bytes, with fp32
PSUM accumulation everywhere. The all-f32 variant is available via
_compute="f32".
"""

import sys

sys.path.insert(0, "/opt/trn_rl_repo")

import numpy as np

N_CORES = 8
B, S, D = 2, 2048, 1024
H, DK = 16, 64
T = (B * S) // N_CORES      # 512 tokens per core
P = 128
KO = D // P                 # 8 contraction tiles
MO = D // P                 # 8 d_out tiles
NC_CHUNKS = T // P          # 4 token chunks per core
HP = H // 2                 # 8 head pairs
NV = D // T                 # 2 output column halves of 512

_cache = {}


def _build_program(compute):
    import concourse.tile as tile
    from concourse import bacc, mybir

    f32 = mybir.dt.float32
    dtc = {"f32": f32, "f16": mybir.dt.float16, "bf16": mybir.dt.bfloat16}[compute]

    nc = bacc.Bacc("TRN2", target_bir_lowering=False, debug=False)

    xq_d = nc.dram_tensor("xq", [D, T], dtc, kind="ExternalInput").ap()
    xk_d = nc.dram_tensor("xk", [D, T], dtc, kind="ExternalInput").ap()
    xv_d = nc.dram_tensor("xv", [D, T], dtc, kind="ExternalInput").ap()
    wq_d = nc.dram_tensor("wq", [MO, P, KO, P], dtc, kind="ExternalInput").ap()
    wk_d = nc.dram_tensor("wk", [MO, P, KO, P], dtc, kind="ExternalInput").ap()
    wv_d = nc.dram_tensor("wv", [D, D], dtc, kind="ExternalInput").ap()
    wo_d = nc.dram_tensor("wo", [D, D], dtc, kind="ExternalInput").ap()
    bq_d = nc.dram_tensor("bq", [P, MO], f32, kind="ExternalInput").ap()
    bk_d = nc.dram_tensor("bk", [P, MO], f32, kind="ExternalInput").ap()
    bv_d = nc.dram_tensor("bv", [D], f32, kind="ExternalInput").ap()
    bo_d = nc.dram_tensor("bo", [D], f32, kind="ExternalInput").ap()
    y_d = nc.dram_tensor("y", [T, D], f32, kind="ExternalOutput").ap()

    with tile.TileContext(nc) as tc:
        with (
            tc.tile_pool(name="singles", bufs=1) as singles,
            tc.tile_pool(name="p2", bufs=4) as p2_pool,
            tc.tile_pool(name="rec", bufs=3) as rec_pool,
            tc.tile_pool(name="ystage", bufs=3) as y_pool,
            tc.tile_pool(name="psproj", bufs=2, space="PSUM") as psproj,
            tc.tile_pool(name="pss", bufs=3, space="PSUM") as pss_pool,
            tc.tile_pool(name="psro", bufs=3, space="PSUM") as psro_pool,
        ):
            # ---- persistent SBUF tensors (inputs as separate per-chunk
            # tiles so each matmul depends only on its own DMA) ----
            xq_t = [singles.tile([P, T], dtc, tag=f"xq{i}", name=f"xq{i}") for i in range(KO)]
            xk_t = [singles.tile([P, T], dtc, tag=f"xk{i}", name=f"xk{i}") for i in range(KO)]
            xv_t = [singles.tile([P, T], dtc, tag=f"xv{i}", name=f"xv{i}") for i in range(KO)]
            wq_t = [singles.tile([P, KO, P], dtc, tag=f"wq{i}", name=f"wq{i}") for i in range(MO)]
            wk_t = [singles.tile([P, KO, P], dtc, tag=f"wk{i}", name=f"wk{i}") for i in range(MO)]
            wv_t = [singles.tile([P, D], dtc, tag=f"wv{i}", name=f"wv{i}") for i in range(KO)]
            wo_t = [singles.tile([P, D], dtc, tag=f"wo{i}", name=f"wo{i}") for i in range(KO)]
            qT_sb = singles.tile([P, MO, T], dtc, tag="qT")
            kT_sb = singles.tile([P, MO, T], dtc, tag="kT")
            v_sb = singles.tile([P, NC_CHUNKS, D], dtc, tag="v")
            oT_sb = singles.tile([P, MO, T], dtc, tag="oT")
            bq_sb = singles.tile([P, MO], f32, tag="bq")
            bk_sb = singles.tile([P, MO], f32, tag="bk")
            bv_sb = singles.tile([P, D], f32, tag="bv")
            bo_sb = singles.tile([P, D], f32, tag="bo")
            ones_sb = singles.tile([P, 64], dtc, tag="ones")

            # PE warm-up: junk matmuls during the DMA lead-in so the HAM
            # clock gate is already at full rate when projections start
            nc.vector.memset(ones_sb[:], 1.0)
            ps_w = psro_pool.tile([64, 64], f32, tag="psro", name="warmup")
            for _ in range(220):
                nc.tensor.matmul(ps_w[:], ones_sb[:, 0:64], ones_sb[:, 0:64],
                                 start=True, stop=True)

            # inputs arrive as flat [128, *] chunks, alternating issue
            # queues, ordered by first use so the projections start early
            nc.sync.dma_start(bq_sb[:], bq_d[:])
            nc.gpsimd.dma_start(bk_sb[:], bk_d[:])
            for ko in range(KO):
                eng = nc.sync if ko % 2 == 0 else nc.gpsimd
                eng.dma_start(xq_t[ko][:], xq_d[ko * P : (ko + 1) * P, :])
                eng.dma_start(
                    wq_t[ko][:].rearrange("p ko c -> p (ko c)"),
                    wq_d[ko].rearrange("p ko c -> p (ko c)"),
                )
            for ko in range(KO):
                eng = nc.sync if ko % 2 == 0 else nc.gpsimd
                eng.dma_start(xk_t[ko][:], xk_d[ko * P : (ko + 1) * P, :])
                eng.dma_start(
                    wk_t[ko][:].rearrange("p ko c -> p (ko c)"),
                    wk_d[ko].rearrange("p ko c -> p (ko c)"),
                )
            for ko in range(KO):
                eng = nc.sync if ko % 2 == 0 else nc.gpsimd
                eng.dma_start(xv_t[ko][:], xv_d[ko * P : (ko + 1) * P, :])
                eng.dma_start(wv_t[ko][:], wv_d[ko * P : (ko + 1) * P, :])
            nc.sync.dma_start(bv_sb[:], bv_d[None, :].to_broadcast([P, D]))
            for ko in range(KO):
                eng = nc.sync if ko % 2 == 0 else nc.gpsimd
                eng.dma_start(wo_t[ko][:], wo_d[ko * P : (ko + 1) * P, :])
            nc.gpsimd.dma_start(bo_sb[:], bo_d[None, :].to_broadcast([P, D]))


            # ---- Q^T / K^T projections (feature-major out) ----
            for w_t, x_t, b_sb, dst in (
                (wq_t, xq_t, bq_sb, qT_sb),
                (wk_t, xk_t, bk_sb, kT_sb),
            ):
                for m in range(MO):
                    ps = psproj.tile([P, T], f32, tag="psproj")
                    for ko in range(KO):
                        nc.tensor.matmul(
                            ps[:],
                            w_t[m][:, ko, :],
                            x_t[ko][:],
                            start=(ko == 0),
                            stop=(ko == KO - 1),
                        )
                    nc.scalar.activation(
                        dst[:, m, :],
                        ps[:],
                        mybir.ActivationFunctionType.Identity,
                        bias=b_sb[:, m : m + 1],
                    )

            # ---- V projection for chunk 0 (later chunks pipeline into the
            # attention loop below to keep the tensor engine dense) ----
            def v_proj_part(mt, part):
                # part 0..7: half n = part//4, contraction pair (2k, 2k+1);
                # one psum tile alive at a time
                n, k2 = part // 4, 2 * (part % 4)
                if part % 4 == 0:
                    v_ps[mt] = psproj.tile(
                        [P, T], f32, tag="psproj", name=f"psv_{mt}_{n}"
                    )
                for ko in (k2, k2 + 1):
                    nc.tensor.matmul(
                        v_ps[mt][:],
                        xv_t[ko][:, mt * P : (mt + 1) * P],
                        wv_t[ko][:, n * T : (n + 1) * T],
                        start=(ko == 0 or ko == k2 == 0 or part % 4 == 0 and ko == k2),
                        stop=(part % 4 == 3 and ko == k2 + 1),
                    )
                if part % 4 == 3:
                    nc.vector.tensor_add(
                        v_sb[:, mt, n * T : (n + 1) * T],
                        v_ps[mt][:],
                        bv_sb[:, n * T : (n + 1) * T],
                    )
                    v_ps[mt] = None

            def y_proj_part(c, part):
                n, k2 = part // 4, 2 * (part % 4)
                if part % 4 == 0:
                    y_ps[c] = psproj.tile(
                        [P, T], f32, tag="psproj", name=f"psy_{c}_{n}"
                    )
                for m in (k2, k2 + 1):
                    nc.tensor.matmul(
                        y_ps[c][:],
                        oT_sb[:, m, c * P : (c + 1) * P],
                        wo_t[m][:, n * T : (n + 1) * T],
                        start=(part % 4 == 0 and m == k2),
                        stop=(part % 4 == 3 and m == k2 + 1),
                    )
                if part % 4 == 3:
                    y_sb = y_pool.tile([P, T], f32, tag="ystage")
                    nc.vector.tensor_add(
                        y_sb[:], y_ps[c][:], bo_sb[:, n * T : (n + 1) * T]
                    )
                    nc.sync.dma_start(
                        y_d[c * P : (c + 1) * P, n * T : (n + 1) * T], y_sb[:]
                    )
                    y_ps[c] = None

            v_ps = [None] * NC_CHUNKS
            y_ps = [None] * NC_CHUNKS
            for part in range(KO):
                v_proj_part(0, part)

            # ---- attention + interleaved output projection, per token chunk ----
            LAG = 5  # iterations between producing an oT tile and consuming it
            iters = [(c, hp) for c in range(NC_CHUNKS) for hp in range(HP)]
            for c in range(NC_CHUNKS):
                tsl = slice(c * P, (c + 1) * P)
                for hp in range(HP):
                    h0, h1 = 2 * hp, 2 * hp + 1
                    p2 = p2_pool.tile([P, 2 * P], dtc, tag="p2")
                    # zero off-diagonal quadrants (exp only writes diagonals)
                    nc.gpsimd.memset(p2[:], 0.0)
                    for idx, h in ((0, h0), (1, h1)):
                        s = slice((h % 2) * 64, (h % 2) * 64 + 64)
                        ps_s = pss_pool.tile([P, P], f32, tag="pss")
                        nc.tensor.matmul(
                            ps_s[:],
                            kT_sb[s, hp, tsl],
                            qT_sb[s, hp, tsl],
                            start=True,
                            stop=True,
                        )
                        for q in (0, 1):
                            qs = slice(q * 64, (q + 1) * 64)
                            nc.scalar.activation(
                                p2[qs, idx * P + q * 64 : idx * P + (q + 1) * 64],
                                ps_s[qs, qs],
                                mybir.ActivationFunctionType.Exp,
                                scale=0.125,
                            )
                    # replicated in-block column sums: head h0 -> partitions
                    # 0:64, head h1 -> partitions 64:128
                    ps_r = psro_pool.tile([P, P], f32, tag="psro")
                    nc.tensor.matmul(
                        ps_r[0:64, :], ones_sb[:], p2[:, 0:P], start=True, stop=True
                    )
                    nc.tensor.matmul(
                        ps_r[64:128, :],
                        ones_sb[:],
                        p2[:, P : 2 * P],
                        start=True,
                        stop=True,
                    )
                    rec = rec_pool.tile([P, P], f32, tag="rec")
                    nc.vector.reciprocal_approx_fast(out=rec[:], in_=ps_r[:])
                    ps_o = psro_pool.tile([P, P], f32, tag="psro")
                    for idx, h in ((0, h0), (1, h1)):
                        nc.tensor.matmul(
                            ps_o[idx * 64 : (idx + 1) * 64, :],
                            v_sb[:, c, h * DK : (h + 1) * DK],
                            p2[:, idx * P : (idx + 1) * P],
                            start=True,
                            stop=True,
                        )
                    nc.vector.tensor_mul(oT_sb[:, hp, tsl], ps_o[:], rec[:])

                    # pipelined dense matmul work: the output-projection
                    # accumulation step for the oT tile produced LAG
                    # iterations ago, and V for chunk c+1
                    k = c * HP + hp
                    if k >= LAG:
                        y_proj_part(*iters[k - LAG])
                    if c + 1 < NC_CHUNKS:
                        v_proj_part(c + 1, hp)

            for k in range(len(iters) - LAG, len(iters)):
                y_proj_part(*iters[k])

    nc.compile()
    return nc


def _get_program(compute):
    if compute not in _cache:
        _cache[compute] = _build_program(compute)
    return _cache[compute]


DEFAULT_COMPUTE = "f16"


def kernel(
    query,
    key,
    value,
    Wq,
    bq,
    Wk,
    bk,
    Wv,
    bv,
    Wo,
    bo,
    _compute=DEFAULT_COMPUTE,
    _trace=False,
):
    from concourse.bass_utils import run_bass_kernel_spmd

    nc = _get_program(_compute)
    if _compute == "bf16":
        import ml_dtypes

        npdt = ml_dtypes.bfloat16
    else:
        npdt = {"f32": np.float32, "f16": np.float16}[_compute]

    def pre_w(w):
        # [din, dout] -> [m, p, ko, c] tiles so each m-tile DMAs contiguously
        return np.ascontiguousarray(
            np.asarray(w, np.float32)
            .reshape(KO, P, MO, P)
            .transpose(2, 1, 0, 3)
            .astype(npdt)
        )

    def pre_b(b):
        return np.ascontiguousarray(np.asarray(b, np.float32).reshape(MO, P).T)

    q2 = np.asarray(query, np.float32).reshape(B * S, D)
    k2 = np.asarray(key, np.float32).reshape(B * S, D)
    v2 = np.asarray(value, np.float32).reshape(B * S, D)
    shared = {
        "wq": pre_w(Wq),
        "wk": pre_w(Wk),
        "wv": np.ascontiguousarray(np.asarray(Wv, np.float32).astype(npdt)),
        "wo": np.ascontiguousarray(np.asarray(Wo, np.float32).astype(npdt)),
        "bq": pre_b(bq),
        "bk": pre_b(bk),
        "bv": np.ascontiguousarray(np.asarray(bv, np.float32)),
        "bo": np.ascontiguousarray(np.asarray(bo, np.float32)),
    }
    in_maps = []
    for c in range(N_CORES):
        rows = slice(c * T, (c + 1) * T)
        in_maps.append(
            {
                "xq": np.ascontiguousarray(q2[rows].T.astype(npdt)),
                "xk": np.ascontiguousarray(k2[rows].T.astype(npdt)),
                "xv": np.ascontiguousarray(v2[rows].T.astype(npdt)),
                **shared,
            }
        )

    kwargs = {}
    if _trace:
        kwargs = {"trace": True}
    res = run_bass_kernel_spmd(nc, in_maps, core_ids=list(range(N_CORES)), **kwargs)
    y = np.concatenate([res.results[c]["y"] for c in range(N_CORES)], axis=0)
    out = y.reshape(B, S, D)
    if _trace:
        return out, res
    return out
